# revision 10
# baseline (speedup 1.0000x reference)
"""Trainium2 Bass kernel for BiDAF-style bidirectional attention.

Reference math (per batch b):
    sim[c,q]  = q[q]·wq + c[c]·wc + sum_e wm[e]*question[q,e]*context[c,e]
    c2q[c,:]  = softmax_q(sim[c,:]) @ question          # (C, E)
    q2c[:]    = softmax_c(max_q sim[c,:]) @ context     # (E,)
    out[c,:]  = [context | c2q | context*c2q | context*q2c]

Sharding: pure data parallel over batch (B=16 -> 2 batches per core x 8 cores).

v2 pipeline (fp16 compute, f32 I/O):
  - all loads emitted up front on the sync HWDGE queue, followed by the
    copy-through stores of cols 0:E (verbatim context); the queue then has
    ~24us of transfer queued before the first dependent store wait.
  - pass A (per pair of 128-row context tiles): fp16 shadow cast of the
    group (gpsimd) -> PE transpose (fp16) -> fp16 sim matmul at N=130 with
    wc folded in as col 128 -> fused add+rowmax on DVE (tensor_tensor_
    reduce emits -(sim+qw) and its min = -rowmax) -> exp on scalar with
    accum_out giving the softmax row-sum for free.
  - pass B (lag 3): fp16 attention-weight transpose (4-deep PSUM ring) ->
    fp16 c2q matmul (N=256, no ones column needed) -> reciprocal of the
    stashed row-sums -> scalar copy-act rescale into cols E:2E -> fused
    (c2q_raw * 1/s) * ctx product straight out of PSUM into cols 2E:3E via
    DVE scalar_tensor_tensor -> store cols E:3E on the sync queue.
  - q2c epilogue per batch right after its pass A drains; ctx*q2c products
    (fp16 x fp16) on gpsimd, stores on the scalar HWDGE queue.
"""

import numpy as np

import concourse.bass as bass
import concourse.tile as tile
import concourse.mybir as mybir
from concourse import bacc
from concourse.bass_utils import run_bass_kernel_spmd
from concourse.masks import make_identity

B, C, Q, E = 16, 2048, 128, 256
NCORES = 8
BPC = B // NCORES          # batches per core
NT = C // 128              # context tiles per batch
NG = NT // 4               # groups of 4 tiles
NP = NT // 2               # pairs per batch
F32 = mybir.dt.float32
F16 = mybir.dt.float16
AX = mybir.AxisListType.X
EXP = mybir.ActivationFunctionType.Exp
CPY = mybir.ActivationFunctionType.Copy
MUL = mybir.AluOpType.mult
ADD = mybir.AluOpType.add
MIN = mybir.AluOpType.min
LAG = 3
import os
# tensor_tensor_reduce crashes/hangs on HW in this usage (in0=PSUM,
# scale=-1, op1=min) — keep the 2-op fallback. scalar_tensor_tensor works.
USE_TTR = os.environ.get("K_TTR", "0") == "1"
USE_STT = os.environ.get("K_STT", "1") == "1"


class _Ctx:
    pass


def _body(tc, out_ext, ctx_in, q_in, wq_in, wc_in, wm_in):
    nc = tc.nc
    with (
        tc.tile_pool(name="singles", bufs=1) as singles,
        tc.tile_pool(name="stgp", bufs=BPC * NG) as stgp,
        tc.tile_pool(name="xc16p", bufs=BPC * NG) as xc16p,
        tc.tile_pool(name="qside", bufs=2) as qside,
        tc.tile_pool(name="work", bufs=4) as work,
        tc.tile_pool(name="pers", bufs=2 * NP) as pers,
        tc.tile_pool(name="statsp", bufs=2) as statsp,
        tc.tile_pool(name="ps_xct", bufs=2, space="PSUM") as ps_xct,
        tc.tile_pool(name="ps_sim", bufs=2, space="PSUM") as ps_sim,
        tc.tile_pool(name="ps_pt", bufs=1, space="PSUM") as ps_pt,
        tc.tile_pool(name="ps_c2q", bufs=1, space="PSUM") as ps_c2q,
        tc.tile_pool(name="ps_misc", bufs=1, space="PSUM") as ps_misc,
    ):
        # ---- constants + params ------------------------------------------
        ident = singles.tile([128, 128], F32)
        make_identity(nc, ident)
        ident_h = singles.tile([128, 128], F16)
        make_identity(nc, ident_h)
        ones_r = singles.tile([1, 128], F32)
        nc.vector.memset(ones_r, 1.0)
        ones_c = singles.tile([128, 1], F32)
        nc.vector.memset(ones_c, 1.0)
        wq_sb = singles.tile([128, 2], F32)
        nc.sync.dma_start(out=wq_sb, in_=wq_in.rearrange("(j p) -> p j", p=128))
        wc_sb = singles.tile([128, 2], F32)
        nc.sync.dma_start(out=wc_sb, in_=wc_in.rearrange("(j p) -> p j", p=128))
        wm_sb = singles.tile([128, 2], F32)
        nc.sync.dma_start(out=wm_sb, in_=wm_in.rearrange("(j p) -> p j", p=128))

        # ---- all loads up front + copy-through of cols 0:E ---------------
        bs = []
        for b in range(BPC):
            st = _Ctx()
            bs.append(st)
            st.qm = qside.tile([128, E], F32, tag="qm", name="qm")
            nc.sync.dma_start(out=st.qm, in_=q_in[b])
        for b in range(BPC):
            st = bs[b]
            st.stgs = []
            for g in range(NG):
                stg = stgp.tile([128, 4, 4 * E], F32, tag="stg", name="stg")
                st.stgs.append(stg)
                nc.sync.dma_start(
                    out=stg[:, :, 0:E],
                    in_=ctx_in[b, g * 512 : (g + 1) * 512, :].rearrange(
                        "(t p) e -> p t e", p=128
                    ),
                )
        for b in range(BPC):
            st = bs[b]
            for g in range(NG):
                # out[:, :, 0:E] is exactly the context: stream it out now
                nc.sync.dma_start(
                    out=out_ext[b, g * 512 : (g + 1) * 512, 0:E].rearrange(
                        "(t p) f -> p t f", p=128
                    ),
                    in_=st.stgs[g][:, :, 0:E],
                )

        # ---- question-side prep for both batches -------------------------
        for b in range(BPC):
            st = bs[b]
            qm = st.qm
            qmt_ps = ps_xct.tile([128, E], F32, tag="xct", name="qmt_ps")
            for j in range(2):
                nc.tensor.transpose(
                    qmt_ps[:, j * 128 : (j + 1) * 128],
                    qm[:, j * 128 : (j + 1) * 128],
                    ident,
                )
            qmt_sb = qside.tile([128, E], F32, tag="qmt", name="qmt_sb")
            nc.vector.tensor_copy(out=qmt_sb, in_=qmt_ps)
            # fp16 question for the c2q matmul
            st.qm16 = qside.tile([128, E], F16, tag="qm16", name="qm16")
            nc.vector.tensor_copy(out=st.qm16, in_=qm)
            # rhs_aug[:, j, 0:128] = wm-chunk * QmT-chunk ; [:, j, 128] = wc
            # col 129 is zero pad to keep the PE output width even (N=130).
            st.rhs_aug = qside.tile([128, 2, 130], F16, tag="rhs_aug",
                                    name="rhs_aug")
            for j in range(2):
                nc.vector.tensor_scalar_mul(
                    st.rhs_aug[:, j, 0:128],
                    qmt_sb[:, j * 128 : (j + 1) * 128],
                    wm_sb[:, j : j + 1],
                )
                nc.vector.tensor_copy(
                    out=st.rhs_aug[:, j, 128:129], in_=wc_sb[:, j : j + 1]
                )
                nc.vector.memset(st.rhs_aug[:, j, 129:130], 0.0)
            qw_ps = ps_misc.tile([1, 128], F32, tag="misc", name="qw_ps")
            for j in range(2):
                nc.tensor.matmul(
                    qw_ps,
                    wq_sb[:, j : j + 1],
                    qmt_sb[:, j * 128 : (j + 1) * 128],
                    start=(j == 0),
                    stop=(j == 1),
                )
            qw_row = qside.tile([1, 128], F32, tag="qw_row", name="qw_row")
            nc.vector.tensor_copy(out=qw_row, in_=qw_ps)
            qwb_ps = ps_misc.tile([128, 128], F32, tag="misc", name="qwb_ps")
            nc.tensor.matmul(qwb_ps, ones_r, qw_row, start=True, stop=True)
            st.qwb2 = qside.tile([128, 2, 128], F32, tag="qwb2", name="qwb2")
            nc.vector.tensor_copy(out=st.qwb2[:, 0, :], in_=qwb_ps)
            nc.vector.tensor_copy(out=st.qwb2[:, 1, :], in_=qwb_ps)
            st.mstat = statsp.tile([128, NT], F32, tag="mstat", name="mstat")
            st.s_all = statsp.tile([128, NP, 2], F32, tag="s_all", name="s_all")
            st.r_all = statsp.tile([128, NP, 2], F32, tag="r_all", name="r_all")
            st.p_sbs = {}
            st.xc16s = []

        # ---- pass A: sim + softmax stats + exp ---------------------------
        def pass_a(b, k):
            st = bs[b]
            g, h = k // 2, k % 2
            stg = st.stgs[g]
            if h == 0:
                xc16 = xc16p.tile([128, 4, E], F16, tag="xc16", name="xc16")
                nc.gpsimd.tensor_copy(out=xc16, in_=stg[:, :, 0:E])
                st.xc16s.append(xc16)
            xc16 = st.xc16s[g]
            xct_ps = ps_xct.tile([128, 2, E], F16, tag="xct", name="xct_ps")
            for i in range(2):
                for j in range(2):
                    nc.tensor.transpose(
                        xct_ps[:, i, j * 128 : (j + 1) * 128],
                        xc16[:, 2 * h + i, j * 128 : (j + 1) * 128],
                        ident_h,
                    )
            xct_sb = work.tile([128, 2, E], F16, tag="xct_sb", name="xct_sb")
            nc.vector.tensor_copy(out=xct_sb, in_=xct_ps)
            sim_ps = ps_sim.tile([128, 2, 130], F32, tag="sim", name="sim_ps")
            for i in range(2):
                for j in range(2):
                    nc.tensor.matmul(
                        sim_ps[:, i, :],
                        xct_sb[:, i, j * 128 : (j + 1) * 128],
                        st.rhs_aug[:, j, :],
                        start=(j == 0),
                        stop=(j == 1),
                    )
            # nsim = -(sim + qw); nm = min(nsim) = -rowmax  (fused on DVE)
            nsim = work.tile([128, 2, 128], F32, tag="nsim", name="nsim")
            nm = work.tile([128, 2], F32, tag="nm", name="nm")
            if USE_TTR:
                for i in range(2):
                    nc.vector.tensor_tensor_reduce(
                        out=nsim[:, i, :],
                        in0=sim_ps[:, i, 0:128],
                        in1=st.qwb2[:, i, :],
                        scale=-1.0,
                        scalar=3.0e38,
                        op0=ADD,
                        op1=MIN,
                        accum_out=nm[:, i : i + 1],
                    )
            else:
                # fallback: sim_in kept positive, nm = -rowmax via negate;
                # the exp then runs with scale=+1 (v1 style).
                nc.vector.tensor_add(nsim, sim_ps[:, :, 0:128], st.qwb2)
                nc.vector.reduce_max(out=nm, in_=nsim, axis=AX, negate=True)
            # mstat = cwc + rowmax = cwc - nm
            nc.vector.tensor_sub(
                st.mstat[:, 2 * k : 2 * k + 2], sim_ps[:, :, 128], nm
            )
            p_sb = pers.tile([128, 2, 128], F16, tag="p_sb", name="p_sb")
            for i in range(2):
                # p = exp(-nsim + nm) = exp(sim + qw - rowmax); row-sum freed
                # into s_all by the activation accumulator.
                nc.scalar.activation(
                    out=p_sb[:, i, :],
                    in_=nsim[:, i, :],
                    func=EXP,
                    bias=nm[:, i : i + 1],
                    scale=-1.0 if USE_TTR else 1.0,
                    accum_out=st.s_all[:, k, i : i + 1],
                )
            st.p_sbs[k] = p_sb

        # ---- pass B: c2q + ctx*c2q + store cols E:3E ---------------------
        def pass_b(b, k):
            st = bs[b]
            g, h = k // 2, k % 2
            stg = st.stgs[g]
            xc16 = st.xc16s[g]
            p_sb = st.p_sbs[k]
            pt_ps = st.pt_ring[:, k % 4, :, :]
            for i in range(2):
                nc.tensor.transpose(pt_ps[:, i, :], p_sb[:, i, :], ident_h)
            pt_sb = work.tile([128, 2, 128], F16, tag="pt_sb", name="pt_sb")
            nc.vector.tensor_copy(out=pt_sb, in_=pt_ps)
            nc.vector.reciprocal(
                out=st.r_all[:, k, :], in_=st.s_all[:, k, :]
            )
            for i in range(2):
                c2q_ps = st.c2q_ring[:, k % 2, i, :]
                nc.tensor.matmul(
                    c2q_ps, pt_sb[:, i, :], st.qm16, start=True, stop=True
                )
                nc.scalar.activation(
                    out=stg[:, 2 * h + i, E : 2 * E],
                    in_=c2q_ps,
                    func=CPY,
                    scale=st.r_all[:, k, i : i + 1],
                )
                if k % 2 == 0 and USE_STT:
                    # ctx*c2q = (c2q_raw * 1/s) * ctx, straight out of PSUM
                    # (DVE only: gpsimd has no PSUM access)
                    nc.vector.scalar_tensor_tensor(
                        out=stg[:, 2 * h + i, 2 * E : 3 * E],
                        in0=c2q_ps,
                        scalar=st.r_all[:, k, i : i + 1],
                        in1=xc16[:, 2 * h + i, :],
                        op0=MUL,
                        op1=MUL,
                    )
                elif k % 2 == 0:
                    nc.vector.tensor_mul(
                        stg[:, 2 * h + i, 2 * E : 3 * E],
                        stg[:, 2 * h + i, E : 2 * E],
                        xc16[:, 2 * h + i, :],
                    )
            if k % 2 == 1:
                # odd pairs: SBUF-side product on gpsimd off the rescaled c2q
                nc.gpsimd.tensor_mul(
                    stg[:, 2 * h : 2 * h + 2, 2 * E : 3 * E],
                    stg[:, 2 * h : 2 * h + 2, E : 2 * E],
                    xc16[:, 2 * h : 2 * h + 2, :],
                )
            r0 = g * 512 + h * 256
            nc.sync.dma_start(
                out=out_ext[b, r0 : r0 + 256, E : 3 * E].rearrange(
                    "(t p) f -> p t f", p=128
                ),
                in_=stg[:, 2 * h : 2 * h + 2, E : 3 * E],
            )

        # ---- q2c epilogue: softmax over C, broadcast weights -------------
        def ep_pre(b):
            st = bs[b]
            mstat = st.mstat
            r1 = statsp.tile([128, 1], F32, tag="r1", name="r1")
            nc.vector.reduce_max(out=r1, in_=mstat, axis=AX)
            r1t_ps = ps_misc.tile([1, 128], F32, tag="misc", name="r1t_ps")
            nc.tensor.transpose(r1t_ps, r1, ident)
            neg_gmax = statsp.tile([1, 1], F32, tag="gmax", name="neg_gmax")
            nc.vector.reduce_max(
                out=neg_gmax, in_=r1t_ps, axis=AX, negate=True
            )
            ngb_ps = ps_misc.tile([128, 1], F32, tag="misc", name="ngb_ps")
            nc.tensor.matmul(ngb_ps, ones_r, neg_gmax, start=True, stop=True)
            ngb_sb = statsp.tile([128, 1], F32, tag="ngb", name="ngb_sb")
            nc.vector.tensor_copy(out=ngb_sb, in_=ngb_ps)
            st.e_sb = statsp.tile([128, NT], F16, tag="e_sb", name="e_sb")
            s_col = statsp.tile([128, 1], F32, tag="s_col", name="s_col")
            nc.scalar.activation(
                out=st.e_sb, in_=mstat, func=EXP, bias=ngb_sb, scale=1.0,
                accum_out=s_col,
            )
            tot_ps = ps_misc.tile([1, 1], F32, tag="misc", name="tot_ps")
            nc.tensor.matmul(tot_ps, s_col, ones_c, start=True, stop=True)
            st.rt_sb = statsp.tile([1, 1], F32, tag="rt", name="rt_sb")
            nc.vector.reciprocal(out=st.rt_sb, in_=tot_ps)

        def ep_q2c(b, half):
            st = bs[b]
            if half == 0:
                st.q2c_ps = ps_misc.tile([1, E], F32, tag="misc",
                                         name="q2c_ps")
            for t in range(half * NT // 2, (half + 1) * NT // 2):
                nc.tensor.matmul(
                    st.q2c_ps,
                    st.e_sb[:, t : t + 1],
                    st.xc16s[t // 4][:, t % 4, :],
                    start=(t == 0),
                    stop=(t == NT - 1),
                )

        def ep_fin(b):
            st = bs[b]
            q2c_sb = statsp.tile([1, E], F32, tag="q2c_sb", name="q2c_sb")
            nc.scalar.activation(
                out=q2c_sb, in_=st.q2c_ps, func=CPY, scale=st.rt_sb
            )
            q2cb_ps = ps_misc.tile([128, E], F32, tag="misc", name="q2cb_ps")
            nc.tensor.matmul(q2cb_ps, ones_r, q2c_sb, start=True, stop=True)
            st.q2cb16 = statsp.tile([128, 2, E], F16, tag="q2cb", name="q2cb16")
            nc.vector.tensor_copy(out=st.q2cb16[:, 0, :], in_=q2cb_ps)
            nc.vector.tensor_copy(out=st.q2cb16[:, 1, :], in_=q2cb_ps)

        # ---- ctx * q2c + store cols 3E:4E --------------------------------
        def stage3(b, g):
            st = bs[b]
            stg = st.stgs[g]
            xc16 = st.xc16s[g]
            for h in range(2):
                nc.gpsimd.tensor_mul(
                    stg[:, 2 * h : 2 * h + 2, 3 * E : 4 * E],
                    xc16[:, 2 * h : 2 * h + 2, :],
                    st.q2cb16,
                )
            nc.scalar.dma_start(
                out=out_ext[
                    b, g * 512 : (g + 1) * 512, 3 * E : 4 * E
                ].rearrange("(t p) f -> p t f", p=128),
                in_=stg[:, :, 3 * E : 4 * E],
            )

        # ---- schedule ----------------------------------------------------
        for b in range(BPC):
            bs[b].pt_ring = ps_pt.tile(
                [128, 4, 2, 128], F16, tag="pt", name="pt_ring"
            )
            bs[b].c2q_ring = ps_c2q.tile(
                [128, 2, 2, E], F32, tag="c2q", name="c2q_ring"
            )
        # Modulo schedule over global pair index kk = b*NP + k.  Pass B lags
        # pass A by LAG pairs; pass B is emitted first inside each round
        # (its inputs are oldest, hence ready).  The q2c epilogue chain for
        # each batch starts right after that batch's pass A drains and is
        # split into small pieces so it never parks mid-queue in front of
        # ready pass-B work.
        TOT = BPC * NP
        for r in range(TOT + LAG + NG + 1):
            if r >= LAG and r - LAG < TOT:
                kk = r - LAG
                pass_b(kk // NP, kk % NP)
            if r < TOT:
                pass_a(r // NP, r % NP)
            if r == NP:
                ep_pre(0)
            elif r == NP + 1:
                ep_q2c(0, 0)
            elif r == NP + 2:
                ep_q2c(0, 1)
            elif r == NP + 3:
                ep_fin(0)
            elif NP + 4 <= r < NP + 4 + 2 * NG and (r - NP) % 2 == 0:
                stage3(0, (r - (NP + 4)) // 2)
            if r == TOT:
                ep_pre(1)
            elif r == TOT + 1:
                ep_q2c(1, 0)
                ep_q2c(1, 1)
            elif r == TOT + 2:
                ep_fin(1)
            elif TOT + 3 <= r < TOT + 3 + NG:
                stage3(1, r - (TOT + 3))


_NC_CACHE = None


def _build():
    global _NC_CACHE
    if _NC_CACHE is not None:
        return _NC_CACHE
    nc = bacc.Bacc(
        "TRN2", target_bir_lowering=False, debug=False, num_devices=NCORES
    )
    ctx_in = nc.dram_tensor("context", [BPC, C, E], F32, kind="ExternalInput").ap()
    q_in = nc.dram_tensor("question", [BPC, Q, E], F32, kind="ExternalInput").ap()
    wq_in = nc.dram_tensor("w_question", [E], F32, kind="ExternalInput").ap()
    wc_in = nc.dram_tensor("w_context", [E], F32, kind="ExternalInput").ap()
    wm_in = nc.dram_tensor("w_multiple", [E], F32, kind="ExternalInput").ap()
    out_ext = nc.dram_tensor("out", [BPC, C, 4 * E], F32, kind="ExternalOutput").ap()
    with tile.TileContext(nc) as tc:
        _body(tc, out_ext, ctx_in, q_in, wq_in, wc_in, wm_in)
    nc.compile()
    _NC_CACHE = nc
    return nc


def _run(inputs, trace=False, **kw):
    nc = _build()
    context = np.ascontiguousarray(np.asarray(inputs["context"], dtype=np.float32))
    question = np.ascontiguousarray(np.asarray(inputs["question"], dtype=np.float32))
    wq = np.ascontiguousarray(np.asarray(inputs["w_question"], dtype=np.float32))
    wc = np.ascontiguousarray(np.asarray(inputs["w_context"], dtype=np.float32))
    wm = np.ascontiguousarray(np.asarray(inputs["w_multiple"], dtype=np.float32))
    in_maps = []
    for i in range(NCORES):
        sl = slice(i * BPC, (i + 1) * BPC)
        in_maps.append(
            {
                "context": context[sl],
                "question": question[sl],
                "w_question": wq,
                "w_context": wc,
                "w_multiple": wm,
            }
        )
    res = run_bass_kernel_spmd(
        nc, in_maps, core_ids=list(range(NCORES)), trace=trace, **kw
    )
    out = np.concatenate([res.results[i]["out"] for i in range(NCORES)], axis=0)
    return out, res


def kernel(**inputs):
    try:
        out, _ = _run(inputs, trace=False)
    except Exception:
        # transient device errors (e.g. a wedged core from a prior run)
        # usually clear on retry
        out, _ = _run(inputs, trace=False)
    return out


# revision 12
# speedup vs baseline: 1.0318x; 1.0318x over previous
"""Trainium2 Bass kernel for BiDAF-style bidirectional attention.

Reference math (per batch b):
    sim[c,q]  = q[q]·wq + c[c]·wc + sum_e wm[e]*question[q,e]*context[c,e]
    c2q[c,:]  = softmax_q(sim[c,:]) @ question          # (C, E)
    q2c[:]    = softmax_c(max_q sim[c,:]) @ context     # (E,)
    out[c,:]  = [context | c2q | context*c2q | context*q2c]

Sharding: pure data parallel over batch (B=16 -> 2 batches per core x 8 cores).

v2 pipeline (fp16 compute, f32 I/O):
  - all loads emitted up front on the sync HWDGE queue, followed by the
    copy-through stores of cols 0:E (verbatim context); the queue then has
    ~24us of transfer queued before the first dependent store wait.
  - pass A (per pair of 128-row context tiles): fp16 shadow cast of the
    group (gpsimd) -> PE transpose (fp16) -> fp16 sim matmul at N=130 with
    wc folded in as col 128 -> fused add+rowmax on DVE (tensor_tensor_
    reduce emits -(sim+qw) and its min = -rowmax) -> exp on scalar with
    accum_out giving the softmax row-sum for free.
  - pass B (lag 3): fp16 attention-weight transpose (4-deep PSUM ring) ->
    fp16 c2q matmul (N=256, no ones column needed) -> reciprocal of the
    stashed row-sums -> scalar copy-act rescale into cols E:2E -> fused
    (c2q_raw * 1/s) * ctx product straight out of PSUM into cols 2E:3E via
    DVE scalar_tensor_tensor -> store cols E:3E on the sync queue.
  - q2c epilogue per batch right after its pass A drains; ctx*q2c products
    (fp16 x fp16) on gpsimd, stores on the scalar HWDGE queue.
"""

import numpy as np

import concourse.bass as bass
import concourse.tile as tile
import concourse.mybir as mybir
from concourse import bacc
from concourse.bass_utils import run_bass_kernel_spmd
from concourse.masks import make_identity

B, C, Q, E = 16, 2048, 128, 256
NCORES = 8
BPC = B // NCORES          # batches per core
NT = C // 128              # context tiles per batch
NG = NT // 4               # groups of 4 tiles
NP = NT // 2               # pairs per batch
F32 = mybir.dt.float32
F16 = mybir.dt.float16
AX = mybir.AxisListType.X
EXP = mybir.ActivationFunctionType.Exp
CPY = mybir.ActivationFunctionType.Copy
MUL = mybir.AluOpType.mult
ADD = mybir.AluOpType.add
MIN = mybir.AluOpType.min
LAG = 3
import os
# tensor_tensor_reduce crashes/hangs on HW in this usage (in0=PSUM,
# scale=-1, op1=min) — keep the 2-op fallback. scalar_tensor_tensor works.
USE_TTR = os.environ.get("K_TTR", "0") == "1"
USE_STT = os.environ.get("K_STT", "1") == "1"


class _Ctx:
    pass


def _body(tc, out_ext, ctx_in, q_in, wq_in, wc_in, wm_in):
    nc = tc.nc
    with (
        tc.tile_pool(name="singles", bufs=1) as singles,
        tc.tile_pool(name="stgp", bufs=BPC * NG) as stgp,
        tc.tile_pool(name="xc16p", bufs=BPC * NG) as xc16p,
        tc.tile_pool(name="qside", bufs=2) as qside,
        tc.tile_pool(name="work", bufs=4) as work,
        tc.tile_pool(name="pers", bufs=2 * NP) as pers,
        tc.tile_pool(name="statsp", bufs=2) as statsp,
        tc.tile_pool(name="ps_xct", bufs=2, space="PSUM") as ps_xct,
        tc.tile_pool(name="ps_sim", bufs=2, space="PSUM") as ps_sim,
        tc.tile_pool(name="ps_pt", bufs=1, space="PSUM") as ps_pt,
        tc.tile_pool(name="ps_c2q", bufs=1, space="PSUM") as ps_c2q,
        tc.tile_pool(name="ps_misc", bufs=1, space="PSUM") as ps_misc,
    ):
        # ---- constants + params ------------------------------------------
        ident = singles.tile([128, 128], F32)
        make_identity(nc, ident)
        ident_h = singles.tile([128, 128], F16)
        make_identity(nc, ident_h)
        ones_r = singles.tile([1, 128], F32)
        nc.vector.memset(ones_r, 1.0)
        ones_c = singles.tile([128, 1], F32)
        nc.vector.memset(ones_c, 1.0)
        wq_sb = singles.tile([128, 2], F32)
        nc.sync.dma_start(out=wq_sb, in_=wq_in.rearrange("(j p) -> p j", p=128))
        wc_sb = singles.tile([128, 2], F32)
        nc.sync.dma_start(out=wc_sb, in_=wc_in.rearrange("(j p) -> p j", p=128))
        wm_sb = singles.tile([128, 2], F32)
        nc.sync.dma_start(out=wm_sb, in_=wm_in.rearrange("(j p) -> p j", p=128))

        # ---- all loads up front + copy-through of cols 0:E ---------------
        bs = []
        for b in range(BPC):
            st = _Ctx()
            bs.append(st)
            st.qm = qside.tile([128, E], F32, tag="qm", name="qm")
            nc.sync.dma_start(out=st.qm, in_=q_in[b])
        for b in range(BPC):
            st = bs[b]
            st.stgs = []
            for g in range(NG):
                stg = stgp.tile([128, 4, 4 * E], F32, tag="stg", name="stg")
                st.stgs.append(stg)
                nc.sync.dma_start(
                    out=stg[:, :, 0:E],
                    in_=ctx_in[b, g * 512 : (g + 1) * 512, :].rearrange(
                        "(t p) e -> p t e", p=128
                    ),
                )
        for b in range(BPC):
            st = bs[b]
            for g in range(NG):
                # out[:, :, 0:E] is exactly the context: stream it out now
                nc.sync.dma_start(
                    out=out_ext[b, g * 512 : (g + 1) * 512, 0:E].rearrange(
                        "(t p) f -> p t f", p=128
                    ),
                    in_=st.stgs[g][:, :, 0:E],
                )

        # ---- question-side prep for both batches -------------------------
        for b in range(BPC):
            st = bs[b]
            qm = st.qm
            qmt_ps = ps_xct.tile([128, E], F32, tag="xct", name="qmt_ps")
            for j in range(2):
                nc.tensor.transpose(
                    qmt_ps[:, j * 128 : (j + 1) * 128],
                    qm[:, j * 128 : (j + 1) * 128],
                    ident,
                )
            qmt_sb = qside.tile([128, E], F32, tag="qmt", name="qmt_sb")
            nc.vector.tensor_copy(out=qmt_sb, in_=qmt_ps)
            # fp16 question for the c2q matmul
            st.qm16 = qside.tile([128, E], F16, tag="qm16", name="qm16")
            nc.vector.tensor_copy(out=st.qm16, in_=qm)
            # rhs_aug[:, j, 0:128] = wm-chunk * QmT-chunk ; [:, j, 128] = wc
            # col 129 is zero pad to keep the PE output width even (N=130).
            st.rhs_aug = qside.tile([128, 2, 130], F16, tag="rhs_aug",
                                    name="rhs_aug")
            for j in range(2):
                nc.vector.tensor_scalar_mul(
                    st.rhs_aug[:, j, 0:128],
                    qmt_sb[:, j * 128 : (j + 1) * 128],
                    wm_sb[:, j : j + 1],
                )
                nc.vector.tensor_copy(
                    out=st.rhs_aug[:, j, 128:129], in_=wc_sb[:, j : j + 1]
                )
                nc.vector.memset(st.rhs_aug[:, j, 129:130], 0.0)
            qw_ps = ps_misc.tile([1, 128], F32, tag="misc", name="qw_ps")
            for j in range(2):
                nc.tensor.matmul(
                    qw_ps,
                    wq_sb[:, j : j + 1],
                    qmt_sb[:, j * 128 : (j + 1) * 128],
                    start=(j == 0),
                    stop=(j == 1),
                )
            qw_row = qside.tile([1, 128], F32, tag="qw_row", name="qw_row")
            nc.vector.tensor_copy(out=qw_row, in_=qw_ps)
            qwb_ps = ps_misc.tile([128, 128], F32, tag="misc", name="qwb_ps")
            nc.tensor.matmul(qwb_ps, ones_r, qw_row, start=True, stop=True)
            st.qwb2 = qside.tile([128, 2, 128], F32, tag="qwb2", name="qwb2")
            nc.vector.tensor_copy(out=st.qwb2[:, 0, :], in_=qwb_ps)
            nc.vector.tensor_copy(out=st.qwb2[:, 1, :], in_=qwb_ps)
            st.mstat = statsp.tile([128, NT], F32, tag="mstat", name="mstat")
            st.s_all = statsp.tile([128, NP, 2], F32, tag="s_all", name="s_all")
            st.r_all = statsp.tile([128, NP, 2], F32, tag="r_all", name="r_all")
            st.p_sbs = {}
            st.xc16s = []

        # ---- pass A: sim + softmax stats + exp ---------------------------
        def pass_a(b, k):
            st = bs[b]
            g, h = k // 2, k % 2
            stg = st.stgs[g]
            if h == 0:
                xc16 = xc16p.tile([128, 4, E], F16, tag="xc16", name="xc16")
                nc.scalar.copy(out=xc16, in_=stg[:, :, 0:E])
                st.xc16s.append(xc16)
            xc16 = st.xc16s[g]
            xct_ps = ps_xct.tile([128, 2, E], F16, tag="xct", name="xct_ps")
            for i in range(2):
                for j in range(2):
                    nc.tensor.transpose(
                        xct_ps[:, i, j * 128 : (j + 1) * 128],
                        xc16[:, 2 * h + i, j * 128 : (j + 1) * 128],
                        ident_h,
                    )
            xct_sb = work.tile([128, 2, E], F16, tag="xct_sb", name="xct_sb")
            nc.vector.tensor_copy(out=xct_sb, in_=xct_ps)
            sim_ps = ps_sim.tile([128, 2, 130], F32, tag="sim", name="sim_ps")
            for i in range(2):
                for j in range(2):
                    nc.tensor.matmul(
                        sim_ps[:, i, :],
                        xct_sb[:, i, j * 128 : (j + 1) * 128],
                        st.rhs_aug[:, j, :],
                        start=(j == 0),
                        stop=(j == 1),
                    )
            # nsim = -(sim + qw); nm = min(nsim) = -rowmax  (fused on DVE)
            nsim = work.tile([128, 2, 128], F32, tag="nsim", name="nsim")
            nm = work.tile([128, 2], F32, tag="nm", name="nm")
            if USE_TTR:
                for i in range(2):
                    nc.vector.tensor_tensor_reduce(
                        out=nsim[:, i, :],
                        in0=sim_ps[:, i, 0:128],
                        in1=st.qwb2[:, i, :],
                        scale=-1.0,
                        scalar=3.0e38,
                        op0=ADD,
                        op1=MIN,
                        accum_out=nm[:, i : i + 1],
                    )
            else:
                # fallback: sim_in kept positive, nm = -rowmax via negate;
                # the exp then runs with scale=+1 (v1 style).
                nc.vector.tensor_add(nsim, sim_ps[:, :, 0:128], st.qwb2)
                nc.vector.reduce_max(out=nm, in_=nsim, axis=AX, negate=True)
            # mstat = cwc + rowmax = cwc - nm
            nc.vector.tensor_sub(
                st.mstat[:, 2 * k : 2 * k + 2], sim_ps[:, :, 128], nm
            )
            p_sb = pers.tile([128, 2, 128], F16, tag="p_sb", name="p_sb")
            for i in range(2):
                # p = exp(-nsim + nm) = exp(sim + qw - rowmax); row-sum freed
                # into s_all by the activation accumulator.
                nc.scalar.activation(
                    out=p_sb[:, i, :],
                    in_=nsim[:, i, :],
                    func=EXP,
                    bias=nm[:, i : i + 1],
                    scale=-1.0 if USE_TTR else 1.0,
                    accum_out=st.s_all[:, k, i : i + 1],
                )
            st.p_sbs[k] = p_sb

        # ---- pass B: c2q + ctx*c2q + store cols E:3E ---------------------
        def pass_b(b, k):
            st = bs[b]
            g, h = k // 2, k % 2
            stg = st.stgs[g]
            xc16 = st.xc16s[g]
            p_sb = st.p_sbs[k]
            pt_ps = st.pt_ring[:, k % 4, :, :]
            for i in range(2):
                nc.tensor.transpose(pt_ps[:, i, :], p_sb[:, i, :], ident_h)
            pt_sb = work.tile([128, 2, 128], F16, tag="pt_sb", name="pt_sb")
            nc.vector.tensor_copy(out=pt_sb, in_=pt_ps)
            nc.vector.reciprocal(
                out=st.r_all[:, k, :], in_=st.s_all[:, k, :]
            )
            for i in range(2):
                c2q_ps = st.c2q_ring[:, k % 2, i, :]
                nc.tensor.matmul(
                    c2q_ps, pt_sb[:, i, :], st.qm16, start=True, stop=True
                )
                nc.scalar.activation(
                    out=stg[:, 2 * h + i, E : 2 * E],
                    in_=c2q_ps,
                    func=CPY,
                    scale=st.r_all[:, k, i : i + 1],
                )
                if k % 2 == 0 and USE_STT:
                    # ctx*c2q = (c2q_raw * 1/s) * ctx, straight out of PSUM
                    # (DVE only: gpsimd has no PSUM access)
                    nc.vector.scalar_tensor_tensor(
                        out=stg[:, 2 * h + i, 2 * E : 3 * E],
                        in0=c2q_ps,
                        scalar=st.r_all[:, k, i : i + 1],
                        in1=xc16[:, 2 * h + i, :],
                        op0=MUL,
                        op1=MUL,
                    )
                elif k % 2 == 0:
                    nc.vector.tensor_mul(
                        stg[:, 2 * h + i, 2 * E : 3 * E],
                        stg[:, 2 * h + i, E : 2 * E],
                        xc16[:, 2 * h + i, :],
                    )
            if k % 2 == 1:
                # odd pairs: SBUF-side product on gpsimd off the rescaled c2q
                nc.gpsimd.tensor_mul(
                    stg[:, 2 * h : 2 * h + 2, 2 * E : 3 * E],
                    stg[:, 2 * h : 2 * h + 2, E : 2 * E],
                    xc16[:, 2 * h : 2 * h + 2, :],
                )
            r0 = g * 512 + h * 256
            nc.sync.dma_start(
                out=out_ext[b, r0 : r0 + 256, E : 3 * E].rearrange(
                    "(t p) f -> p t f", p=128
                ),
                in_=stg[:, 2 * h : 2 * h + 2, E : 3 * E],
            )

        # ---- q2c epilogue: softmax over C, broadcast weights -------------
        def ep_pre(b):
            st = bs[b]
            mstat = st.mstat
            r1 = statsp.tile([128, 1], F32, tag="r1", name="r1")
            nc.vector.reduce_max(out=r1, in_=mstat, axis=AX)
            r1t_ps = ps_misc.tile([1, 128], F32, tag="misc", name="r1t_ps")
            nc.tensor.transpose(r1t_ps, r1, ident)
            neg_gmax = statsp.tile([1, 1], F32, tag="gmax", name="neg_gmax")
            nc.vector.reduce_max(
                out=neg_gmax, in_=r1t_ps, axis=AX, negate=True
            )
            ngb_ps = ps_misc.tile([128, 1], F32, tag="misc", name="ngb_ps")
            nc.tensor.matmul(ngb_ps, ones_r, neg_gmax, start=True, stop=True)
            ngb_sb = statsp.tile([128, 1], F32, tag="ngb", name="ngb_sb")
            nc.vector.tensor_copy(out=ngb_sb, in_=ngb_ps)
            st.e_sb = statsp.tile([128, NT], F16, tag="e_sb", name="e_sb")
            s_col = statsp.tile([128, 1], F32, tag="s_col", name="s_col")
            nc.scalar.activation(
                out=st.e_sb, in_=mstat, func=EXP, bias=ngb_sb, scale=1.0,
                accum_out=s_col,
            )
            tot_ps = ps_misc.tile([1, 1], F32, tag="misc", name="tot_ps")
            nc.tensor.matmul(tot_ps, s_col, ones_c, start=True, stop=True)
            st.rt_sb = statsp.tile([1, 1], F32, tag="rt", name="rt_sb")
            nc.vector.reciprocal(out=st.rt_sb, in_=tot_ps)

        def ep_q2c(b, half):
            st = bs[b]
            if half == 0:
                st.q2c_ps = ps_misc.tile([1, E], F32, tag="misc",
                                         name="q2c_ps")
            for t in range(half * NT // 2, (half + 1) * NT // 2):
                nc.tensor.matmul(
                    st.q2c_ps,
                    st.e_sb[:, t : t + 1],
                    st.xc16s[t // 4][:, t % 4, :],
                    start=(t == 0),
                    stop=(t == NT - 1),
                )

        def ep_fin(b):
            st = bs[b]
            q2c_sb = statsp.tile([1, E], F32, tag="q2c_sb", name="q2c_sb")
            nc.scalar.activation(
                out=q2c_sb, in_=st.q2c_ps, func=CPY, scale=st.rt_sb
            )
            q2cb_ps = ps_misc.tile([128, E], F32, tag="misc", name="q2cb_ps")
            nc.tensor.matmul(q2cb_ps, ones_r, q2c_sb, start=True, stop=True)
            st.q2cb16 = statsp.tile([128, 2, E], F16, tag="q2cb", name="q2cb16")
            nc.vector.tensor_copy(out=st.q2cb16[:, 0, :], in_=q2cb_ps)
            nc.vector.tensor_copy(out=st.q2cb16[:, 1, :], in_=q2cb_ps)

        # ---- ctx * q2c + store cols 3E:4E --------------------------------
        def stage3(b, g):
            st = bs[b]
            stg = st.stgs[g]
            xc16 = st.xc16s[g]
            for h in range(2):
                eng = nc.vector if h == 0 else nc.gpsimd
                eng.tensor_mul(
                    stg[:, 2 * h : 2 * h + 2, 3 * E : 4 * E],
                    xc16[:, 2 * h : 2 * h + 2, :],
                    st.q2cb16,
                )
            nc.gpsimd.dma_start(
                out=out_ext[
                    b, g * 512 : (g + 1) * 512, 3 * E : 4 * E
                ].rearrange("(t p) f -> p t f", p=128),
                in_=stg[:, :, 3 * E : 4 * E],
            )

        # ---- schedule ----------------------------------------------------
        for b in range(BPC):
            bs[b].pt_ring = ps_pt.tile(
                [128, 4, 2, 128], F16, tag="pt", name="pt_ring"
            )
            bs[b].c2q_ring = ps_c2q.tile(
                [128, 2, 2, E], F32, tag="c2q", name="c2q_ring"
            )
        # Modulo schedule over global pair index kk = b*NP + k.  Pass B lags
        # pass A by LAG pairs; pass B is emitted first inside each round
        # (its inputs are oldest, hence ready).  The q2c epilogue chain for
        # each batch starts right after that batch's pass A drains and is
        # split into small pieces so it never parks mid-queue in front of
        # ready pass-B work.
        TOT = BPC * NP
        for r in range(TOT + LAG + NG + 1):
            if r >= LAG and r - LAG < TOT:
                kk = r - LAG
                pass_b(kk // NP, kk % NP)
            if r < TOT:
                pass_a(r // NP, r % NP)
            if r == NP:
                ep_pre(0)
            elif r == NP + 1:
                ep_q2c(0, 0)
            elif r == NP + 2:
                ep_q2c(0, 1)
            elif r == NP + 3:
                ep_fin(0)
            elif NP + 4 <= r < NP + 4 + 2 * NG and (r - NP) % 2 == 0:
                stage3(0, (r - (NP + 4)) // 2)
            if r == TOT:
                ep_pre(1)
            elif r == TOT + 1:
                ep_q2c(1, 0)
                ep_q2c(1, 1)
            elif r == TOT + 2:
                ep_fin(1)
            elif TOT + 3 <= r < TOT + 3 + NG:
                stage3(1, r - (TOT + 3))


_NC_CACHE = None


def _build():
    global _NC_CACHE
    if _NC_CACHE is not None:
        return _NC_CACHE
    nc = bacc.Bacc(
        "TRN2", target_bir_lowering=False, debug=False, num_devices=NCORES
    )
    ctx_in = nc.dram_tensor("context", [BPC, C, E], F32, kind="ExternalInput").ap()
    q_in = nc.dram_tensor("question", [BPC, Q, E], F32, kind="ExternalInput").ap()
    wq_in = nc.dram_tensor("w_question", [E], F32, kind="ExternalInput").ap()
    wc_in = nc.dram_tensor("w_context", [E], F32, kind="ExternalInput").ap()
    wm_in = nc.dram_tensor("w_multiple", [E], F32, kind="ExternalInput").ap()
    out_ext = nc.dram_tensor("out", [BPC, C, 4 * E], F32, kind="ExternalOutput").ap()
    with tile.TileContext(nc) as tc:
        _body(tc, out_ext, ctx_in, q_in, wq_in, wc_in, wm_in)
    nc.compile()
    _NC_CACHE = nc
    return nc


def _run(inputs, trace=False, **kw):
    nc = _build()
    context = np.ascontiguousarray(np.asarray(inputs["context"], dtype=np.float32))
    question = np.ascontiguousarray(np.asarray(inputs["question"], dtype=np.float32))
    wq = np.ascontiguousarray(np.asarray(inputs["w_question"], dtype=np.float32))
    wc = np.ascontiguousarray(np.asarray(inputs["w_context"], dtype=np.float32))
    wm = np.ascontiguousarray(np.asarray(inputs["w_multiple"], dtype=np.float32))
    in_maps = []
    for i in range(NCORES):
        sl = slice(i * BPC, (i + 1) * BPC)
        in_maps.append(
            {
                "context": context[sl],
                "question": question[sl],
                "w_question": wq,
                "w_context": wc,
                "w_multiple": wm,
            }
        )
    res = run_bass_kernel_spmd(
        nc, in_maps, core_ids=list(range(NCORES)), trace=trace, **kw
    )
    out = np.concatenate([res.results[i]["out"] for i in range(NCORES)], axis=0)
    return out, res


def kernel(**inputs):
    try:
        out, _ = _run(inputs, trace=False)
    except Exception:
        # transient device errors (e.g. a wedged core from a prior run)
        # usually clear on retry
        out, _ = _run(inputs, trace=False)
    return out


# revision 16
# speedup vs baseline: 1.0593x; 1.0266x over previous
"""Trainium2 Bass kernel for BiDAF-style bidirectional attention.

Reference math (per batch b):
    sim[c,q]  = q[q]·wq + c[c]·wc + sum_e wm[e]*question[q,e]*context[c,e]
    c2q[c,:]  = softmax_q(sim[c,:]) @ question          # (C, E)
    q2c[:]    = softmax_c(max_q sim[c,:]) @ context     # (E,)
    out[c,:]  = [context | c2q | context*c2q | context*q2c]

Sharding: pure data parallel over batch (B=16 -> 2 batches per core x 8 cores).

v2 pipeline (fp16 compute, f32 I/O):
  - all loads emitted up front on the sync HWDGE queue, followed by the
    copy-through stores of cols 0:E (verbatim context); the queue then has
    ~24us of transfer queued before the first dependent store wait.
  - pass A (per pair of 128-row context tiles): fp16 shadow cast of the
    group (gpsimd) -> PE transpose (fp16) -> fp16 sim matmul at N=130 with
    wc folded in as col 128 -> fused add+rowmax on DVE (tensor_tensor_
    reduce emits -(sim+qw) and its min = -rowmax) -> exp on scalar with
    accum_out giving the softmax row-sum for free.
  - pass B (lag 3): fp16 attention-weight transpose (4-deep PSUM ring) ->
    fp16 c2q matmul (N=256, no ones column needed) -> reciprocal of the
    stashed row-sums -> scalar copy-act rescale into cols E:2E -> fused
    (c2q_raw * 1/s) * ctx product straight out of PSUM into cols 2E:3E via
    DVE scalar_tensor_tensor -> store cols E:3E on the sync queue.
  - q2c epilogue per batch right after its pass A drains; ctx*q2c products
    (fp16 x fp16) on gpsimd, stores on the scalar HWDGE queue.
"""

import numpy as np

import concourse.bass as bass
import concourse.tile as tile
import concourse.mybir as mybir
from concourse import bacc
from concourse.bass_utils import run_bass_kernel_spmd
from concourse.masks import make_identity

B, C, Q, E = 16, 2048, 128, 256
NCORES = 8
BPC = B // NCORES          # batches per core
NT = C // 128              # context tiles per batch
NG = NT // 4               # groups of 4 tiles
NP = NT // 2               # pairs per batch
F32 = mybir.dt.float32
F16 = mybir.dt.float16
AX = mybir.AxisListType.X
EXP = mybir.ActivationFunctionType.Exp
CPY = mybir.ActivationFunctionType.Copy
MUL = mybir.AluOpType.mult
ADD = mybir.AluOpType.add
MIN = mybir.AluOpType.min
LAG = 5
import os
# tensor_tensor_reduce crashes/hangs on HW in this usage (in0=PSUM,
# scale=-1, op1=min) — keep the 2-op fallback. scalar_tensor_tensor works.
USE_TTR = os.environ.get("K_TTR", "0") == "1"
USE_STT = os.environ.get("K_STT", "1") == "1"


class _Ctx:
    pass


def _body(tc, out_ext, ctx_in, q_in, wq_in, wc_in, wm_in):
    nc = tc.nc
    with (
        tc.tile_pool(name="singles", bufs=1) as singles,
        tc.tile_pool(name="stgp", bufs=BPC * NG) as stgp,
        tc.tile_pool(name="xc16p", bufs=BPC * NG) as xc16p,
        tc.tile_pool(name="qside", bufs=2) as qside,
        tc.tile_pool(name="work", bufs=4) as work,
        tc.tile_pool(name="pers", bufs=2 * NP) as pers,
        tc.tile_pool(name="statsp", bufs=2) as statsp,
        tc.tile_pool(name="ps_xct", bufs=2, space="PSUM") as ps_xct,
        tc.tile_pool(name="ps_sim", bufs=2, space="PSUM") as ps_sim,
        tc.tile_pool(name="ps_pt", bufs=1, space="PSUM") as ps_pt,
        tc.tile_pool(name="ps_c2q", bufs=1, space="PSUM") as ps_c2q,
        tc.tile_pool(name="ps_misc", bufs=1, space="PSUM") as ps_misc,
    ):
        # ---- constants + params ------------------------------------------
        ident = singles.tile([128, 128], F32)
        make_identity(nc, ident)
        ident_h = singles.tile([128, 128], F16)
        make_identity(nc, ident_h)
        ones_r = singles.tile([1, 128], F32)
        nc.vector.memset(ones_r, 1.0)
        ones_c = singles.tile([128, 1], F32)
        nc.vector.memset(ones_c, 1.0)
        wq_sb = singles.tile([128, 2], F32)
        nc.sync.dma_start(out=wq_sb, in_=wq_in.rearrange("(j p) -> p j", p=128))
        wc_sb = singles.tile([128, 2], F32)
        nc.sync.dma_start(out=wc_sb, in_=wc_in.rearrange("(j p) -> p j", p=128))
        wm_sb = singles.tile([128, 2], F32)
        nc.sync.dma_start(out=wm_sb, in_=wm_in.rearrange("(j p) -> p j", p=128))

        # ---- all loads up front + copy-through of cols 0:E ---------------
        bs = []
        for b in range(BPC):
            st = _Ctx()
            bs.append(st)
            st.qm = qside.tile([128, E], F32, tag="qm", name="qm")
            nc.sync.dma_start(out=st.qm, in_=q_in[b])
        for b in range(BPC):
            st = bs[b]
            st.stgs = []
            for g in range(NG):
                stg = stgp.tile([128, 4, 4 * E], F32, tag="stg", name="stg")
                st.stgs.append(stg)
                nc.sync.dma_start(
                    out=stg[:, :, 0:E],
                    in_=ctx_in[b, g * 512 : (g + 1) * 512, :].rearrange(
                        "(t p) e -> p t e", p=128
                    ),
                )
        def copythru(b, g):
            # out[:, :, 0:E] is exactly the context; issued on the gpsimd
            # SWDGE queue so it drains independently of the sync queue --
            # always-ready filler traffic for any store-readiness stall.
            nc.gpsimd.dma_start(
                out=out_ext[b, g * 512 : (g + 1) * 512, 0:E].rearrange(
                    "(t p) f -> p t f", p=128
                ),
                in_=bs[b].stgs[g][:, :, 0:E],
            )

        # ---- question-side prep for both batches -------------------------
        for b in range(BPC):
            st = bs[b]
            qm = st.qm
            qmt_ps = ps_xct.tile([128, E], F32, tag="xct", name="qmt_ps")
            for j in range(2):
                nc.tensor.transpose(
                    qmt_ps[:, j * 128 : (j + 1) * 128],
                    qm[:, j * 128 : (j + 1) * 128],
                    ident,
                )
            qmt_sb = qside.tile([128, E], F32, tag="qmt", name="qmt_sb")
            nc.vector.tensor_copy(out=qmt_sb, in_=qmt_ps)
            # fp16 question for the c2q matmul
            st.qm16 = qside.tile([128, E], F16, tag="qm16", name="qm16")
            nc.vector.tensor_copy(out=st.qm16, in_=qm)
            # rhs_aug[:, j, 0:128] = wm-chunk * QmT-chunk ; [:, j, 128] = wc
            # col 129 is zero pad to keep the PE output width even (N=130).
            st.rhs_aug = qside.tile([128, 2, 130], F16, tag="rhs_aug",
                                    name="rhs_aug")
            for j in range(2):
                nc.vector.tensor_scalar_mul(
                    st.rhs_aug[:, j, 0:128],
                    qmt_sb[:, j * 128 : (j + 1) * 128],
                    wm_sb[:, j : j + 1],
                )
                nc.vector.tensor_copy(
                    out=st.rhs_aug[:, j, 128:129], in_=wc_sb[:, j : j + 1]
                )
                nc.vector.memset(st.rhs_aug[:, j, 129:130], 0.0)
            qw_ps = ps_misc.tile([1, 128], F32, tag="misc", name="qw_ps")
            for j in range(2):
                nc.tensor.matmul(
                    qw_ps,
                    wq_sb[:, j : j + 1],
                    qmt_sb[:, j * 128 : (j + 1) * 128],
                    start=(j == 0),
                    stop=(j == 1),
                )
            qw_row = qside.tile([1, 128], F32, tag="qw_row", name="qw_row")
            nc.vector.tensor_copy(out=qw_row, in_=qw_ps)
            qwb_ps = ps_misc.tile([128, 128], F32, tag="misc", name="qwb_ps")
            nc.tensor.matmul(qwb_ps, ones_r, qw_row, start=True, stop=True)
            st.qwb2 = qside.tile([128, 2, 128], F32, tag="qwb2", name="qwb2")
            nc.vector.tensor_copy(out=st.qwb2[:, 0, :], in_=qwb_ps)
            nc.vector.tensor_copy(out=st.qwb2[:, 1, :], in_=qwb_ps)
            st.mstat = statsp.tile([128, NT], F32, tag="mstat", name="mstat")
            st.s_all = statsp.tile([128, NP, 2], F32, tag="s_all", name="s_all")
            st.r_all = statsp.tile([128, NP, 2], F32, tag="r_all", name="r_all")
            st.p_sbs = {}
            st.xc16s = []

        # ---- pass A: sim + softmax stats + exp ---------------------------
        def pass_a(b, k):
            st = bs[b]
            g, h = k // 2, k % 2
            stg = st.stgs[g]
            if h == 0:
                xc16 = xc16p.tile([128, 4, E], F16, tag="xc16", name="xc16")
                nc.scalar.copy(out=xc16, in_=stg[:, :, 0:E])
                st.xc16s.append(xc16)
            xc16 = st.xc16s[g]
            xct_ps = ps_xct.tile([128, 2, E], F16, tag="xct", name="xct_ps")
            for i in range(2):
                for j in range(2):
                    nc.tensor.transpose(
                        xct_ps[:, i, j * 128 : (j + 1) * 128],
                        xc16[:, 2 * h + i, j * 128 : (j + 1) * 128],
                        ident_h,
                    )
            xct_sb = work.tile([128, 2, E], F16, tag="xct_sb", name="xct_sb")
            nc.vector.tensor_copy(out=xct_sb, in_=xct_ps)
            sim_ps = ps_sim.tile([128, 2, 130], F32, tag="sim", name="sim_ps")
            for i in range(2):
                for j in range(2):
                    nc.tensor.matmul(
                        sim_ps[:, i, :],
                        xct_sb[:, i, j * 128 : (j + 1) * 128],
                        st.rhs_aug[:, j, :],
                        start=(j == 0),
                        stop=(j == 1),
                    )
            # nsim = -(sim + qw); nm = min(nsim) = -rowmax  (fused on DVE)
            nsim = work.tile([128, 2, 128], F32, tag="nsim", name="nsim")
            nm = work.tile([128, 2], F32, tag="nm", name="nm")
            if USE_TTR:
                for i in range(2):
                    nc.vector.tensor_tensor_reduce(
                        out=nsim[:, i, :],
                        in0=sim_ps[:, i, 0:128],
                        in1=st.qwb2[:, i, :],
                        scale=-1.0,
                        scalar=3.0e38,
                        op0=ADD,
                        op1=MIN,
                        accum_out=nm[:, i : i + 1],
                    )
            else:
                # fallback: sim_in kept positive, nm = -rowmax via negate;
                # the exp then runs with scale=+1 (v1 style).
                nc.vector.tensor_add(nsim, sim_ps[:, :, 0:128], st.qwb2)
                nc.vector.reduce_max(out=nm, in_=nsim, axis=AX, negate=True)
            # mstat = cwc + rowmax = cwc - nm
            nc.vector.tensor_sub(
                st.mstat[:, 2 * k : 2 * k + 2], sim_ps[:, :, 128], nm
            )
            p_sb = pers.tile([128, 2, 128], F16, tag="p_sb", name="p_sb")
            for i in range(2):
                # p = exp(-nsim + nm) = exp(sim + qw - rowmax); row-sum freed
                # into s_all by the activation accumulator.
                nc.scalar.activation(
                    out=p_sb[:, i, :],
                    in_=nsim[:, i, :],
                    func=EXP,
                    bias=nm[:, i : i + 1],
                    scale=-1.0 if USE_TTR else 1.0,
                    accum_out=st.s_all[:, k, i : i + 1],
                )
            st.p_sbs[k] = p_sb

        # ---- pass B: c2q + ctx*c2q + store cols E:3E ---------------------
        def pass_b(b, k):
            st = bs[b]
            g, h = k // 2, k % 2
            stg = st.stgs[g]
            xc16 = st.xc16s[g]
            p_sb = st.p_sbs[k]
            pt_ps = st.pt_ring[:, k % 4, :, :]
            for i in range(2):
                nc.tensor.transpose(pt_ps[:, i, :], p_sb[:, i, :], ident_h)
            pt_sb = work.tile([128, 2, 128], F16, tag="pt_sb", name="pt_sb")
            nc.vector.tensor_copy(out=pt_sb, in_=pt_ps)
            nc.vector.reciprocal(
                out=st.r_all[:, k, :], in_=st.s_all[:, k, :]
            )
            for i in range(2):
                c2q_ps = st.c2q_ring[:, k % 2, i, :]
                nc.tensor.matmul(
                    c2q_ps, pt_sb[:, i, :], st.qm16, start=True, stop=True
                )
                nc.scalar.activation(
                    out=stg[:, 2 * h + i, E : 2 * E],
                    in_=c2q_ps,
                    func=CPY,
                    scale=st.r_all[:, k, i : i + 1],
                )
                if k % 2 == 0 and USE_STT:
                    # ctx*c2q = (c2q_raw * 1/s) * ctx, straight out of PSUM
                    # (DVE only: gpsimd has no PSUM access)
                    nc.vector.scalar_tensor_tensor(
                        out=stg[:, 2 * h + i, 2 * E : 3 * E],
                        in0=c2q_ps,
                        scalar=st.r_all[:, k, i : i + 1],
                        in1=xc16[:, 2 * h + i, :],
                        op0=MUL,
                        op1=MUL,
                    )
                elif k % 2 == 0:
                    nc.vector.tensor_mul(
                        stg[:, 2 * h + i, 2 * E : 3 * E],
                        stg[:, 2 * h + i, E : 2 * E],
                        xc16[:, 2 * h + i, :],
                    )
            if k % 2 == 1:
                # odd pairs: SBUF-side product on gpsimd off the rescaled c2q
                nc.gpsimd.tensor_mul(
                    stg[:, 2 * h : 2 * h + 2, 2 * E : 3 * E],
                    stg[:, 2 * h : 2 * h + 2, E : 2 * E],
                    xc16[:, 2 * h : 2 * h + 2, :],
                )
            r0 = g * 512 + h * 256
            nc.sync.dma_start(
                out=out_ext[b, r0 : r0 + 256, E : 3 * E].rearrange(
                    "(t p) f -> p t f", p=128
                ),
                in_=stg[:, 2 * h : 2 * h + 2, E : 3 * E],
            )

        # ---- q2c epilogue: softmax over C, broadcast weights -------------
        def ep_pre(b):
            st = bs[b]
            mstat = st.mstat
            r1 = statsp.tile([128, 1], F32, tag="r1", name="r1")
            nc.vector.reduce_max(out=r1, in_=mstat, axis=AX)
            r1t_ps = ps_misc.tile([1, 128], F32, tag="misc", name="r1t_ps")
            nc.tensor.transpose(r1t_ps, r1, ident)
            neg_gmax = statsp.tile([1, 1], F32, tag="gmax", name="neg_gmax")
            nc.vector.reduce_max(
                out=neg_gmax, in_=r1t_ps, axis=AX, negate=True
            )
            ngb_ps = ps_misc.tile([128, 1], F32, tag="misc", name="ngb_ps")
            nc.tensor.matmul(ngb_ps, ones_r, neg_gmax, start=True, stop=True)
            ngb_sb = statsp.tile([128, 1], F32, tag="ngb", name="ngb_sb")
            nc.vector.tensor_copy(out=ngb_sb, in_=ngb_ps)
            st.e_sb = statsp.tile([128, NT], F16, tag="e_sb", name="e_sb")
            s_col = statsp.tile([128, 1], F32, tag="s_col", name="s_col")
            nc.scalar.activation(
                out=st.e_sb, in_=mstat, func=EXP, bias=ngb_sb, scale=1.0,
                accum_out=s_col,
            )
            tot_ps = ps_misc.tile([1, 1], F32, tag="misc", name="tot_ps")
            nc.tensor.matmul(tot_ps, s_col, ones_c, start=True, stop=True)
            st.rt_sb = statsp.tile([1, 1], F32, tag="rt", name="rt_sb")
            nc.vector.reciprocal(out=st.rt_sb, in_=tot_ps)

        def ep_q2c(b, half):
            st = bs[b]
            if half == 0:
                st.q2c_ps = ps_misc.tile([1, E], F32, tag="misc",
                                         name="q2c_ps")
            for t in range(half * NT // 2, (half + 1) * NT // 2):
                nc.tensor.matmul(
                    st.q2c_ps,
                    st.e_sb[:, t : t + 1],
                    st.xc16s[t // 4][:, t % 4, :],
                    start=(t == 0),
                    stop=(t == NT - 1),
                )

        def ep_fin(b):
            st = bs[b]
            q2c_sb = statsp.tile([1, E], F32, tag="q2c_sb", name="q2c_sb")
            nc.scalar.activation(
                out=q2c_sb, in_=st.q2c_ps, func=CPY, scale=st.rt_sb
            )
            q2cb_ps = ps_misc.tile([128, E], F32, tag="misc", name="q2cb_ps")
            nc.tensor.matmul(q2cb_ps, ones_r, q2c_sb, start=True, stop=True)
            st.q2cb16 = statsp.tile([128, 2, E], F16, tag="q2cb", name="q2cb16")
            nc.vector.tensor_copy(out=st.q2cb16[:, 0, :], in_=q2cb_ps)
            nc.vector.tensor_copy(out=st.q2cb16[:, 1, :], in_=q2cb_ps)

        # ---- ctx * q2c + store cols 3E:4E --------------------------------
        def stage3(b, g):
            st = bs[b]
            stg = st.stgs[g]
            xc16 = st.xc16s[g]
            for h in range(2):
                # mid-kernel (batch 0) stage3 runs fully on gpsimd (DVE is
                # the loaded engine there); the tail (batch 1) splits V/G.
                eng = nc.vector if (b == 1 and h == 0) else nc.gpsimd
                eng.tensor_mul(
                    stg[:, 2 * h : 2 * h + 2, 3 * E : 4 * E],
                    xc16[:, 2 * h : 2 * h + 2, :],
                    st.q2cb16,
                )
            nc.gpsimd.dma_start(
                out=out_ext[
                    b, g * 512 : (g + 1) * 512, 3 * E : 4 * E
                ].rearrange("(t p) f -> p t f", p=128),
                in_=stg[:, :, 3 * E : 4 * E],
            )

        # ---- schedule ----------------------------------------------------
        for b in range(BPC):
            bs[b].pt_ring = ps_pt.tile(
                [128, 4, 2, 128], F16, tag="pt", name="pt_ring"
            )
            bs[b].c2q_ring = ps_c2q.tile(
                [128, 2, 2, E], F32, tag="c2q", name="c2q_ring"
            )
        # Modulo schedule over global pair index kk = b*NP + k.  Pass B lags
        # pass A by LAG pairs; pass B is emitted first inside each round
        # (its inputs are oldest, hence ready).  The q2c epilogue chain for
        # each batch starts right after that batch's pass A drains and is
        # split into small pieces so it never parks mid-queue in front of
        # ready pass-B work.
        TOT = BPC * NP
        for r in range(TOT + LAG + NG + 1):
            if r >= LAG and r - LAG < TOT:
                kk = r - LAG
                pass_b(kk // NP, kk % NP)
            if r < TOT:
                pass_a(r // NP, r % NP)
            if 1 <= r <= BPC * NG:
                g = r - 1
                copythru(g // NG, g % NG)
            if r == NP:
                ep_pre(0)
            elif r == NP + 1:
                ep_q2c(0, 0)
            elif r == NP + 2:
                ep_q2c(0, 1)
            elif r == NP + 3:
                ep_fin(0)
            elif NP + 4 <= r < NP + 4 + 2 * NG and (r - NP) % 2 == 0:
                stage3(0, (r - (NP + 4)) // 2)
            if r == TOT:
                ep_pre(1)
            elif r == TOT + 1:
                ep_q2c(1, 0)
                ep_q2c(1, 1)
            elif r == TOT + 2:
                ep_fin(1)
            elif TOT + 3 <= r < TOT + 3 + NG:
                stage3(1, r - (TOT + 3))


_NC_CACHE = None


def _build():
    global _NC_CACHE
    if _NC_CACHE is not None:
        return _NC_CACHE
    nc = bacc.Bacc(
        "TRN2", target_bir_lowering=False, debug=False, num_devices=NCORES
    )
    ctx_in = nc.dram_tensor("context", [BPC, C, E], F32, kind="ExternalInput").ap()
    q_in = nc.dram_tensor("question", [BPC, Q, E], F32, kind="ExternalInput").ap()
    wq_in = nc.dram_tensor("w_question", [E], F32, kind="ExternalInput").ap()
    wc_in = nc.dram_tensor("w_context", [E], F32, kind="ExternalInput").ap()
    wm_in = nc.dram_tensor("w_multiple", [E], F32, kind="ExternalInput").ap()
    out_ext = nc.dram_tensor("out", [BPC, C, 4 * E], F32, kind="ExternalOutput").ap()
    with tile.TileContext(nc) as tc:
        _body(tc, out_ext, ctx_in, q_in, wq_in, wc_in, wm_in)
    nc.compile()
    _NC_CACHE = nc
    return nc


def _run(inputs, trace=False, **kw):
    nc = _build()
    context = np.ascontiguousarray(np.asarray(inputs["context"], dtype=np.float32))
    question = np.ascontiguousarray(np.asarray(inputs["question"], dtype=np.float32))
    wq = np.ascontiguousarray(np.asarray(inputs["w_question"], dtype=np.float32))
    wc = np.ascontiguousarray(np.asarray(inputs["w_context"], dtype=np.float32))
    wm = np.ascontiguousarray(np.asarray(inputs["w_multiple"], dtype=np.float32))
    in_maps = []
    for i in range(NCORES):
        sl = slice(i * BPC, (i + 1) * BPC)
        in_maps.append(
            {
                "context": context[sl],
                "question": question[sl],
                "w_question": wq,
                "w_context": wc,
                "w_multiple": wm,
            }
        )
    res = run_bass_kernel_spmd(
        nc, in_maps, core_ids=list(range(NCORES)), trace=trace, **kw
    )
    out = np.concatenate([res.results[i]["out"] for i in range(NCORES)], axis=0)
    return out, res


def kernel(**inputs):
    try:
        out, _ = _run(inputs, trace=False)
    except Exception:
        # transient device errors (e.g. a wedged core from a prior run)
        # usually clear on retry
        out, _ = _run(inputs, trace=False)
    return out
